# revision 16
# baseline (speedup 1.0000x reference)
"""PinGNN Trainium2 kernel — 8-core SPMD Bass implementation (v3).

Strategy (dst-sharded edges, no scatter in the hot path):
  - Per core: 6250-node shard; edges partitioned by dst.
  - Layer-1 neighbor sum  SB[n] = sum_{e: dst=n} B1[src[e]]  via padded
    fixed-K dma_gather from two HBM half-tables (int16 indices, each half's
    structure independently degree-sorted) + in-SBUF tree reductions; the
    second half merges back through a small permute-gather.
  - B1 table built with 3-term bf16 hi/lo matmuls (fp32 stationary loads
    are ~5x slower on the PE).
  - edge_attr pre-aggregated once (EA), each layer's C-term = EA @ Wc.
  - x[dst] term collapses to deg * (x@Wa + b_eff).
  - Layers computed feature-major with weight-stationary matmuls; layer 2
    reduces straight to graph level (gsel/cnt2 matmuls) + one [64,64]
    AllReduce.
"""

import os

import numpy as np
import ml_dtypes

BF16 = ml_dtypes.bfloat16

# --- problem constants (hardcoded; kernel.py must be self-contained) ---
N = 50000
E = 800000
CE = 16
NG = 64
NC = 8
P = 128
SHARD = N // NC             # 6250
NCH = (SHARD + P - 1) // P  # 49
SLOTS = NCH * P             # 6272
HALF = 32256                # = 63 write-groups of 512; also int16-safe
GW = 32                     # column budget per gather bin
MAX_BIN_CH = 8
NT = (SLOTS + 511) // 512   # 13 column-tiles (last one 128 wide)
F32 = np.float32

_CACHE = {}
_DEBUG_DUMPS = False


def _build_debug(plan):
    global _DEBUG_DUMPS
    _DEBUG_DUMPS = True
    try:
        return _build(plan)
    finally:
        _DEBUG_DUMPS = False


# ----------------------------------------------------------------------
# host-side preprocessing
# ----------------------------------------------------------------------

def _pack_idx16(idx32):
    Pp, C = idx32.shape
    assert Pp == P
    w16 = np.transpose(idx32.reshape(8, 16, C), (1, 2, 0)).reshape(16, C * 8)
    return np.tile(w16.astype(np.int16), (8, 1))


def _pack_flat_idx16(flat):
    return np.tile(flat.astype(np.int16).reshape(-1, 16).T, (8, 1))


def _make_bins(Ks):
    bins = []
    c = 0
    n = len(Ks)
    while c < n:
        kq = max(Ks[c], 1)
        nch = 1
        while (
            c + nch < n
            and nch < MAX_BIN_CH
            and (nch + 1) * max(kq, Ks[c + nch], 1) <= GW
        ):
            kq = max(kq, Ks[c + nch], 1)
            nch += 1
        assert kq <= 2 * GW, f"chunk K={kq} too large"
        bins.append((c, nch, kq))
        c += nch
    return bins


def _bin_offsets(bins):
    off = np.zeros(NCH, np.int64)
    o = 0
    for c0, nch, kq in bins:
        for i in range(nch):
            off[c0 + i] = o + i * kq
        o += nch * kq
    return off, int(o)


def _host_prep(inputs):
    x = np.ascontiguousarray(np.asarray(inputs["x"], F32))
    ea_full = np.ascontiguousarray(np.asarray(inputs["edge_attr"], F32))
    ei = np.asarray(inputs["edge_index"]).astype(np.int64)
    gi = np.asarray(inputs["graph_index"]).astype(np.int64)
    src, dst = ei[0], ei[1]

    cores_raw = []
    K0_all = np.zeros((NC, NCH), np.int64)
    K1_all = np.zeros((NC, NCH), np.int64)
    for k in range(NC):
        n0 = k * SHARD
        emask = (dst >= n0) & (dst < n0 + SHARD)
        esrc = src[emask]
        edst = (dst[emask] - n0).astype(np.int64)
        eidx = np.nonzero(emask)[0]
        deg = np.bincount(edst, minlength=SHARD)
        half1 = esrc >= HALF
        deg0 = np.bincount(edst[~half1], minlength=SHARD)
        deg1 = np.bincount(edst[half1], minlength=SHARD)
        pi0 = np.argsort(deg0, kind="stable")
        pi1 = np.argsort(deg1, kind="stable")
        d0s = deg0[pi0]
        d1s = deg1[pi1]
        for c in range(NCH):
            lo, hi = c * P, min((c + 1) * P, SHARD)
            if lo < SHARD:
                K0_all[k, c] = d0s[lo:hi].max()
                K1_all[k, c] = d1s[lo:hi].max()
        cores_raw.append(
            dict(n0=n0, esrc=esrc, edst=edst, eidx=eidx, deg=deg,
                 half1=half1, pi0=pi0, pi1=pi1))

    K0c = np.maximum(K0_all.max(0), 1)
    K1c = np.maximum(K1_all.max(0), 1)
    bins0 = _make_bins(list(K0c))
    bins1 = _make_bins(list(K1c))
    off0, C0 = _bin_offsets(bins0)
    off1, C1 = _bin_offsets(bins1)
    plan = dict(bins0=bins0, bins1=bins1, C0=C0, C1=C1)

    pad0 = HALF          # zero row at end of b1lo
    pad1 = N - HALF      # zero row at end of b1hi

    xT = np.ascontiguousarray(x.T)
    xTh = xT.astype(BF16)
    xTl = (xT - xTh.astype(F32)).astype(BF16)

    gdst = gi[dst]
    cnt_mat = np.zeros((N, NG), F32)
    np.add.at(cnt_mat, (src, gdst), 1.0)

    in_maps = []
    for k, co in enumerate(cores_raw):
        pi0, pi1 = co["pi0"], co["pi1"]
        inv0 = np.empty(SHARD, np.int64)
        inv0[pi0] = np.arange(SHARD)
        inv1 = np.empty(SHARD, np.int64)
        inv1[pi1] = np.arange(SHARD)

        def structure(sel, inv, off, C, idx_map, padv):
            s_src = co["esrc"][sel]
            s_dst = co["edst"][sel]
            s_ei = co["eidx"][sel]
            order = np.argsort(s_dst, kind="stable")
            s_src, s_dst, s_ei = s_src[order], s_dst[order], s_ei[order]
            counts = np.bincount(s_dst, minlength=SHARD)
            starts = np.zeros(SHARD, np.int64)
            np.cumsum(counts[:-1], out=starts[1:])
            rank = np.arange(len(s_dst)) - starts[s_dst]
            slot = inv[s_dst]
            lane = slot % P
            col = off[slot // P] + rank
            idx = np.full((P, C), padv, np.int32)
            idx[lane, col] = idx_map(s_src)
            ealay = np.zeros((P, C, CE), F32)
            ealay[lane, col] = ea_full[s_ei]
            return idx, ealay

        idx0, ea0 = structure(~co["half1"], inv0, off0, C0,
                              lambda s: s.astype(np.int32), pad0)
        idx1, ea1 = structure(co["half1"], inv1, off1, C1,
                              lambda s: (s - HALF).astype(np.int32), pad1)

        permi = np.zeros(SLOTS, np.int64)
        s1 = inv1[pi0]
        permi[:SHARD] = (s1 % P) * NCH + (s1 // P)
        idxp = _pack_flat_idx16(permi)

        def slotify(a):
            out = np.zeros((SLOTS,) + a.shape[1:], a.dtype)
            out[:SHARD] = a[pi0]
            return out

        n0 = co["n0"]
        xpT = np.ascontiguousarray(slotify(x[n0 : n0 + SHARD]).T)
        deg_row = np.ascontiguousarray(
            slotify(co["deg"].astype(F32))[None, :])

        gsel = np.zeros((SLOTS, NG), F32)
        gsel[np.arange(SHARD), gi[n0 : n0 + SHARD][pi0]] = 1.0
        gsel_l = np.ascontiguousarray(gsel.reshape(NCH, P, NG).transpose(1, 0, 2))
        cnt2 = np.zeros((SLOTS, NG), F32)
        cnt2[:SHARD] = cnt_mat[pi0 + n0]
        cnt2_l = np.ascontiguousarray(cnt2.reshape(NCH, P, NG).transpose(1, 0, 2))

        in_maps.append(dict(
            xTh=xTh, xTl=xTl, xpT=xpT, deg_row=deg_row,
            gsel=gsel_l, cnt2=cnt2_l,
            idx0=_pack_idx16(idx0), idx1=_pack_idx16(idx1), idxp=idxp,
            ea0=ea0, ea1=ea1,
        ))

    W1e = np.asarray(inputs["W1_eff"], F32)
    W2e = np.asarray(inputs["W2_eff"], F32)
    W1o = np.ascontiguousarray(np.asarray(inputs["W1_out"], F32))
    W2o = np.ascontiguousarray(np.asarray(inputs["W2_out"], F32))
    W1b = np.ascontiguousarray(W1e[64:128])
    W1bh = W1b.astype(BF16)
    W1bl = (W1b - W1bh.astype(F32)).astype(BF16)
    cnt_g = np.bincount(gi, minlength=NG).astype(F32)
    shared = dict(
        W1a=np.ascontiguousarray(W1e[:64]),
        W1c=np.ascontiguousarray(W1e[128:]),
        W2a=np.ascontiguousarray(W2e[:64]),
        W2b=np.ascontiguousarray(W2e[64:128]),
        W2c=np.ascontiguousarray(W2e[128:]),
        W1out=W1o, W2out=W2o,
        W2ob=np.ascontiguousarray(W2o[64:]),
        W1bh=W1bh, W1bl=W1bl,
        b1e_c=np.ascontiguousarray(np.asarray(inputs["b1_eff"], F32)[:, None]),
        b2e_c=np.ascontiguousarray(np.asarray(inputs["b2_eff"], F32)[:, None]),
        b1o_c=np.ascontiguousarray(np.asarray(inputs["b1_out"], F32)[:, None]),
        b2o_b=np.tile(np.asarray(inputs["b2_out"], F32)[None, :], (NG, 1)),
        cnt_c=np.ascontiguousarray(np.maximum(cnt_g, 1.0)[:, None]),
        cnt_raw=np.ascontiguousarray(cnt_g[:, None]),
    )
    for m in in_maps:
        m.update(shared)
    return plan, in_maps


# ----------------------------------------------------------------------
# bass program
# ----------------------------------------------------------------------

def _build(plan):
    import contextlib

    import concourse.bacc as bacc
    import concourse.mybir as mybir
    import concourse.tile as tile
    from concourse.masks import make_identity

    f32 = mybir.dt.float32
    bf16 = mybir.dt.bfloat16
    i16 = mybir.dt.int16
    AF = mybir.ActivationFunctionType
    OP = mybir.AluOpType
    C0, C1 = plan["C0"], plan["C1"]
    bins0, bins1 = plan["bins0"], plan["bins1"]
    max_cols = max(kq * nch for (_, nch, kq) in bins0 + bins1)
    max_nch = max(nch for (_, nch, kq) in bins0 + bins1)

    nc = bacc.Bacc(None, target_bir_lowering=False, debug=False,
                   num_swdge_queues=4)

    din = {}
    for name, shape, dt in [
        ("xTh", [64, N], bf16), ("xTl", [64, N], bf16),
        ("xpT", [64, SLOTS], f32), ("deg_row", [1, SLOTS], f32),
        ("gsel", [P, NCH, NG], f32), ("cnt2", [P, NCH, NG], f32),
        ("ea0", [P, C0, CE], f32), ("ea1", [P, C1, CE], f32),
        ("W1a", [64, 64], f32), ("W1c", [CE, 64], f32),
        ("W2a", [64, 64], f32), ("W2b", [64, 64], f32), ("W2c", [CE, 64], f32),
        ("W1out", [P, 64], f32), ("W2out", [P, 64], f32),
        ("W2ob", [64, 64], f32),
        ("W1bh", [64, 64], bf16), ("W1bl", [64, 64], bf16),
        ("b1e_c", [64, 1], f32), ("b2e_c", [64, 1], f32),
        ("b1o_c", [64, 1], f32), ("b2o_b", [NG, 64], f32),
        ("cnt_c", [NG, 1], f32), ("cnt_raw", [NG, 1], f32),
        ("idx0", [P, C0 * 8], i16), ("idx1", [P, C1 * 8], i16),
        ("idxp", [P, SLOTS // 16], i16),
    ]:
        din[name] = nc.dram_tensor(name, shape, dt, kind="ExternalInput")

    out_t = nc.dram_tensor("out", [NG, 64], f32, kind="ExternalOutput")
    dbg = {}
    if _DEBUG_DUMPS:
        dbg["SB1"] = nc.dram_tensor("dbg_SB1", [P, NCH * 64], f32,
                                    kind="ExternalOutput")
        dbg["EA"] = nc.dram_tensor("dbg_EA", [P, NCH * 32], f32,
                                   kind="ExternalOutput")
        dbg["h1T"] = nc.dram_tensor("dbg_h1T", [64, SLOTS], f32,
                                    kind="ExternalOutput")
    ar_in = nc.dram_tensor("ar_in", [NG, 64], f32)
    ar_out = nc.dram_tensor("ar_out", [NG, 64], f32, addr_space="Shared")
    b1lo = nc.dram_tensor("b1lo", [HALF + 1, 64], f32)
    b1hi = nc.dram_tensor("b1hi", [N - HALF + 1, 64], f32)
    r1buf = nc.dram_tensor("r1buf", [SLOTS, P], f32)

    with tile.TileContext(nc) as tc, contextlib.ExitStack() as ctx:
        res = ctx.enter_context(tc.tile_pool(name="res", bufs=1))
        xtp = ctx.enter_context(tc.tile_pool(name="xtp", bufs=3))
        b1p = ctx.enter_context(tc.tile_pool(name="b1p", bufs=4))
        gp = ctx.enter_context(tc.tile_pool(name="gp", bufs=3))
        eap = ctx.enter_context(tc.tile_pool(name="eap", bufs=3))
        ixp = ctx.enter_context(tc.tile_pool(name="ixp", bufs=3))
        combp = ctx.enter_context(tc.tile_pool(name="combp", bufs=3))
        rpp = ctx.enter_context(tc.tile_pool(name="rpp", bufs=1))  # 2 tags
        stg = ctx.enter_context(tc.tile_pool(name="stg", bufs=2))
        sml = ctx.enter_context(tc.tile_pool(name="sml", bufs=2))
        gcp = ctx.enter_context(tc.tile_pool(name="gcp", bufs=3))
        psw = ctx.enter_context(tc.tile_pool(name="psw", bufs=4, space="PSUM"))
        psc = ctx.enter_context(tc.tile_pool(name="psc", bufs=2, space="PSUM"))
        psF_p = ctx.enter_context(tc.tile_pool(name="psFp", bufs=1, space="PSUM"))

        # ---- resident small tensors ----
        wsb = {}
        for nm in ["W1a", "W1c", "W2a", "W2b", "W2c", "W1out", "W2out",
                   "W2ob", "W1bh", "W1bl", "b1e_c", "b2e_c", "b1o_c",
                   "b2o_b", "cnt_c", "cnt_raw", "deg_row"]:
            t = res.tile(list(din[nm].shape), din[nm].dtype, name=f"sb_{nm}")
            nc.sync.dma_start(out=t[:], in_=din[nm][:])
            wsb[nm] = t
        idxp_sb = res.tile([P, SLOTS // 16], i16, name="idxp_sb")
        nc.sync.dma_start(out=idxp_sb[:], in_=din["idxp"][:])

        ones_c = res.tile([1, 64], f32, name="ones_c")
        nc.gpsimd.memset(ones_c[:], 1.0)
        ident = res.tile([P, P], f32, name="ident")
        make_identity(nc, ident[:])
        zrow = res.tile([1, 64], f32, name="zrow")
        nc.gpsimd.memset(zrow[:], 0.0)

        SB1 = res.tile([P, NCH * 64], f32, name="SB1")
        EAs = res.tile([P, NCH * 32], f32, name="EAs")
        nc.gpsimd.memset(EAs[:], 0.0)
        EA_T = res.tile([CE, SLOTS], f32, name="EA_T")
        h1T = res.tile([64, SLOTS], f32, name="h1T")
        sb1v = SB1[:].rearrange("p (c f) -> p c f", f=64)
        eav = EAs[:].rearrange("p (c f) -> p c f", f=32)

        # ---- phase B: B1 half-tables (groups of 512 nodes) ----
        nc.sync.dma_start(out=b1lo[HALF : HALF + 1, :], in_=zrow[:])
        nc.sync.dma_start(out=b1hi[N - HALF : N - HALF + 1, :], in_=zrow[:])
        ngrp = (N + 511) // 512  # 98; groups 63.. go to b1hi
        grp_order = list(range(63, ngrp)) + list(range(63))
        for gi_, g in enumerate(grp_order):
            lo = g * 512
            hi = min(lo + 512, N)
            w = hi - lo
            xh = xtp.tile([64, 512], bf16, tag="xh")
            nc.sync.dma_start(out=xh[:, :w], in_=din["xTh"][:, lo:hi])
            xl = xtp.tile([64, 512], bf16, tag="xl")
            nc.sync.dma_start(out=xl[:, :w], in_=din["xTl"][:, lo:hi])
            ps = psw.tile([P, 256], f32, tag="psw")
            nblk = (w + P - 1) // P
            for j in range(nblk):
                rows = min(P, w - j * P)
                sl = ps[:rows, j * 64 : (j + 1) * 64]
                lh = xh[:, j * P : j * P + rows]
                ll = xl[:, j * P : j * P + rows]
                nc.tensor.matmul(out=sl, lhsT=lh, rhs=wsb["W1bh"][:],
                                 start=True, stop=False)
                nc.tensor.matmul(out=sl, lhsT=lh, rhs=wsb["W1bl"][:],
                                 start=False, stop=False)
                nc.tensor.matmul(out=sl, lhsT=ll, rhs=wsb["W1bh"][:],
                                 start=False, stop=True)
            sb = b1p.tile([P, 256], f32, tag="b1sb")
            if gi_ % 2 == 0:
                nc.vector.tensor_copy(out=sb[:], in_=ps[:])
            else:
                nc.scalar.copy(out=sb[:], in_=ps[:])
            sbv = sb[:].rearrange("p (j f) -> p j f", f=64)
            tab, r0 = (b1hi, lo - HALF) if g >= 63 else (b1lo, lo)
            for j in range(nblk):
                rows = min(P, w - j * P)
                nc.sync.dma_start(
                    out=tab[r0 + j * P : r0 + j * P + rows, :],
                    in_=sbv[:rows, j, :])

        # ---- gathers + tree reductions (bins1 first) ----
        qn = [0]

        def tree(view4, kq, dest):
            """Reduce [128, nch, K, f] over K; final level lands in dest."""
            K = kq
            if K == 1:
                nc.vector.tensor_copy(out=dest, in_=view4[:, :, 0, :])
                return
            while K > 2:
                h = K // 2
                nc.vector.tensor_tensor(
                    out=view4[:, :, 0:h, :], in0=view4[:, :, 0:h, :],
                    in1=view4[:, :, K - h : K, :], op=OP.add)
                K -= h
            nc.vector.tensor_tensor(
                out=dest, in0=view4[:, :, 0, :], in1=view4[:, :, 1, :],
                op=OP.add)

        def gather_half(bins, idx_dram, tab_ap, ea_in, to_r1):
            o = 0
            for (c0, nch, kq) in bins:
                cols = nch * kq
                ix = ixp.tile([P, max_cols * 8], i16, tag="ix")
                nc.sync.dma_start(
                    out=ix[:, : cols * 8],
                    in_=idx_dram[:, o * 8 : (o + cols) * 8])
                g = gp.tile([P, max_cols * 64], f32, tag="G")
                g3 = g[:, : cols * 64].rearrange("p (c f) -> p c f", f=64)
                g4 = g[:, : cols * 64].rearrange(
                    "p (c k f) -> p c k f", k=kq, f=64)
                nidx = P * cols
                nc.gpsimd.dma_gather(
                    g3, tab_ap, ix[:, : cols * 8], nidx, nidx, 64,
                    single_packet=False, queue_num=qn[0] % 4)
                qn[0] += 1
                et = eap.tile([P, max_cols * CE], f32, tag="EAt")
                nc.sync.dma_start(
                    out=et[:, : cols * CE],
                    in_=ea_in[:, o : o + cols, :].rearrange("p c f -> p (c f)"))
                ev = et[:, : cols * CE].rearrange(
                    "p (c k f) -> p c k f", k=kq, f=CE)
                if not to_r1:
                    tree(g4, kq, sb1v[:, c0 : c0 + nch, :])
                    tree(ev, kq, eav[:, c0 : c0 + nch, 0:CE])
                else:
                    cb = combp.tile([P, max_nch * P], f32, tag="comb")
                    cbv = cb[:, : nch * P].rearrange("p (c f) -> p c f", f=P)
                    tree(g4, kq, cbv[:, :, 0:64])
                    tree(ev, kq, cbv[:, :, 64 : 64 + CE])
                    nc.sync.dma_start(
                        out=r1buf[:].rearrange(
                            "(p c) f -> p c f", c=NCH)[:, c0 : c0 + nch, :],
                        in_=cbv)
                o += cols

        gather_half(bins1, din["idx1"][:], b1hi[:], din["ea1"][:], to_r1=True)

        # permute-gathers of half-1 results (adds deferred until bins0 done)
        rps = []
        for (c0, c1) in [(0, 25), (25, NCH)]:
            nchh = c1 - c0
            rp = rpp.tile([P, 25 * P], f32, tag=f"rp{c0}")
            rpv = rp[:, : nchh * P].rearrange("p (c f) -> p c f", f=P)
            nidx = nchh * P
            nc.gpsimd.dma_gather(
                rpv, r1buf[:], idxp_sb[:, c0 * 8 : c1 * 8], nidx, nidx, P,
                single_packet=False, queue_num=qn[0] % 4)
            qn[0] += 1
            rps.append((c0, c1, rpv))

        gather_half(bins0, din["idx0"][:], b1lo[:], din["ea0"][:], to_r1=False)

        for (c0, c1, rpv) in rps:
            nc.vector.tensor_tensor(
                out=sb1v[:, c0:c1, :], in0=sb1v[:, c0:c1, :],
                in1=rpv[:, :, 0:64], op=OP.add)
            nc.vector.tensor_tensor(
                out=eav[:, c0:c1, 0:CE], in0=eav[:, c0:c1, 0:CE],
                in1=rpv[:, :, 64 : 64 + CE], op=OP.add)

        if _DEBUG_DUMPS:
            nc.sync.dma_start(out=dbg["SB1"][:], in_=SB1[:])
            nc.sync.dma_start(out=dbg["EA"][:], in_=EAs[:])

        # ---- EA transpose (PE) ----
        for c in range(NCH):
            pt = psc.tile([32, P], f32, tag="psc")
            nc.tensor.transpose(out=pt[:], in_=eav[:, c, :],
                                identity=ident[:])
            nc.vector.tensor_copy(
                out=EA_T[:, c * P : (c + 1) * P], in_=pt[0:CE, :])

        # ---- layer 1 (per 512-col tile, feature-major) ----
        def tile_rng(t):
            lo = t * 512
            return lo, min(lo + 512, SLOTS) - lo

        for t in range(NT):
            lo, w = tile_rng(t)
            cat = stg.tile([P, 512], f32, tag="cat1")
            nc.sync.dma_start(out=cat[0:64, :w], in_=din["xpT"][:, lo : lo + w])
            psA = psw.tile([64, 512], f32, tag="psw")
            nc.tensor.matmul(out=psA[:, :w], lhsT=wsb["W1a"][:],
                             rhs=cat[0:64, :w], start=True, stop=True)
            psE = psw.tile([64, 512], f32, tag="psw")
            nc.tensor.matmul(out=psE[:, :w], lhsT=wsb["W1c"][:],
                             rhs=EA_T[0:CE, lo : lo + w],
                             start=True, stop=True)
            psD = psw.tile([64, 512], f32, tag="psw")
            nc.tensor.matmul(out=psD[:, :w], lhsT=ones_c[:],
                             rhs=wsb["deg_row"][:, lo : lo + w],
                             start=True, stop=True)
            # agg1_T = (A1 + b1e)*deg + EAW1 + SB1_T  -> cat rows 64:128
            tA = stg.tile([64, 512], f32, tag="tA")
            nc.vector.tensor_scalar_add(
                out=tA[:, :w], in0=psA[:, :w], scalar1=wsb["b1e_c"][:])
            nc.vector.tensor_tensor(
                out=tA[:, :w], in0=tA[:, :w], in1=psD[:, :w], op=OP.mult)
            nc.vector.tensor_tensor(
                out=tA[:, :w], in0=tA[:, :w], in1=psE[:, :w], op=OP.add)
            psT = psw.tile([64, 512], f32, tag="psw")
            for j in range((w + P - 1) // P):
                c = (lo + j * P) // P
                nc.tensor.transpose(
                    out=psT[:, j * P : (j + 1) * P], in_=sb1v[:, c, :],
                    identity=ident[:])
            nc.vector.tensor_tensor(
                out=cat[64:128, :w], in0=tA[:, :w], in1=psT[:, :w], op=OP.add)
            psH = psw.tile([64, 512], f32, tag="psw")
            nc.tensor.matmul(out=psH[:, :w], lhsT=wsb["W1out"][:],
                             rhs=cat[:, :w], start=True, stop=True)
            nc.scalar.activation(out=h1T[:, lo : lo + w], in_=psH[:, :w],
                                 func=AF.Relu, bias=wsb["b1o_c"][:])

        if _DEBUG_DUMPS:
            nc.sync.dma_start(out=dbg["h1T"][:], in_=h1T[:])

        # ---- layer 2 ----
        psF = psF_p.tile([NG, 64], f32, name="psF")
        psF2 = psF_p.tile([64, NG], f32, name="psF2")
        gselv = din["gsel"][:]
        cntv = din["cnt2"][:]
        nmm = [0]
        for t in range(NT):
            lo, w = tile_rng(t)
            cat = stg.tile([P, 512], f32, tag="cat2")
            nc.vector.tensor_copy(out=cat[0:64, :w], in_=h1T[:, lo : lo + w])
            psA = psw.tile([64, 512], f32, tag="psw")
            nc.tensor.matmul(out=psA[:, :w], lhsT=wsb["W2a"][:],
                             rhs=h1T[:, lo : lo + w], start=True, stop=True)
            psE = psw.tile([64, 512], f32, tag="psw")
            nc.tensor.matmul(out=psE[:, :w], lhsT=wsb["W2c"][:],
                             rhs=EA_T[0:CE, lo : lo + w],
                             start=True, stop=True)
            psD = psw.tile([64, 512], f32, tag="psw")
            nc.tensor.matmul(out=psD[:, :w], lhsT=ones_c[:],
                             rhs=wsb["deg_row"][:, lo : lo + w],
                             start=True, stop=True)
            tA = stg.tile([64, 512], f32, tag="tA2")
            nc.vector.tensor_scalar_add(
                out=tA[:, :w], in0=psA[:, :w], scalar1=wsb["b2e_c"][:])
            nc.vector.tensor_tensor(
                out=tA[:, :w], in0=tA[:, :w], in1=psD[:, :w], op=OP.mult)
            nc.vector.tensor_tensor(
                out=cat[64:128, :w], in0=tA[:, :w], in1=psE[:, :w], op=OP.add)
            psU = psw.tile([64, 512], f32, tag="psw")
            nc.tensor.matmul(out=psU[:, :w], lhsT=wsb["W2out"][:],
                             rhs=cat[:, :w], start=True, stop=True)
            sbU = stg.tile([64, 512], f32, tag="sbU")
            nc.vector.tensor_copy(out=sbU[:, :w], in_=psU[:, :w])
            # per-chunk: transpose U, node-major matmuls into psF/psF2
            for j in range((w + P - 1) // P):
                c = (lo + j * P) // P
                ptU = psc.tile([P, 64], f32, tag="psc")
                nc.tensor.transpose(
                    out=ptU[:], in_=sbU[:, j * P : (j + 1) * P],
                    identity=ident[0:64, 0:64])
                uc = gcp.tile([P, 64], f32, tag="uc")
                nc.vector.tensor_copy(out=uc[:], in_=ptU[:])
                gc_ = gcp.tile([P, NG], f32, tag="gselc")
                nc.sync.dma_start(out=gc_[:], in_=gselv[:, c, :])
                nc.tensor.matmul(out=psF[:], lhsT=gc_[:], rhs=uc[:],
                                 start=(c == 0), stop=False,
                                 skip_group_check=True)
                psB = psc.tile([P, 64], f32, tag="psc")
                nc.tensor.matmul(out=psB[:], lhsT=h1T[:, c * P : (c + 1) * P],
                                 rhs=wsb["W2b"][:], start=True, stop=True)
                bc = gcp.tile([P, 64], f32, tag="bc")
                nc.vector.tensor_copy(out=bc[:], in_=psB[:])
                cc_ = gcp.tile([P, NG], f32, tag="cntc")
                nc.sync.dma_start(out=cc_[:], in_=cntv[:, c, :])
                nc.tensor.matmul(out=psF2[:], lhsT=bc[:], rhs=cc_[:],
                                 start=(c == 0), stop=(c == NCH - 1),
                                 skip_group_check=True)

        sbS = sml.tile([64, NG], f32, tag="sbS")
        nc.vector.tensor_copy(out=sbS[:], in_=psF2[:])
        nc.tensor.matmul(out=psF[:], lhsT=sbS[:], rhs=wsb["W2ob"][:],
                         start=False, stop=True, skip_group_check=True)
        sbF = sml.tile([NG, 64], f32, tag="sbF")
        nc.vector.tensor_copy(out=sbF[:], in_=psF[:])
        nc.sync.dma_start(out=ar_in[:], in_=sbF[:])
        nc.gpsimd.collective_compute(
            "AllReduce", OP.add, replica_groups=[list(range(NC))],
            ins=[ar_in[:]], outs=[ar_out[:]])
        sbAR = sml.tile([NG, 64], f32, tag="sbAR")
        nc.sync.dma_start(out=sbAR[:], in_=ar_out[:])
        tb = sml.tile([NG, 64], f32, tag="tb")
        nc.vector.tensor_scalar_mul(
            out=tb[:], in0=wsb["b2o_b"][:], scalar1=wsb["cnt_raw"][:])
        nc.vector.tensor_tensor(out=sbAR[:], in0=sbAR[:], in1=tb[:], op=OP.add)
        inv = sml.tile([NG, 1], f32, tag="inv")
        nc.vector.reciprocal(out=inv[:], in_=wsb["cnt_c"][:])
        nc.vector.tensor_scalar_mul(out=sbAR[:], in0=sbAR[:], scalar1=inv[:])
        nc.sync.dma_start(out=out_t[:], in_=sbAR[:])

    nc.finalize()
    return nc


# ----------------------------------------------------------------------
# entry point
# ----------------------------------------------------------------------

def kernel(**inputs) -> np.ndarray:
    plan, in_maps = _host_prep(inputs)
    key = (tuple(plan["bins0"]), tuple(plan["bins1"]), plan["C0"], plan["C1"],
           _DEBUG_DUMPS)
    if _CACHE.get("key") != key:
        _CACHE["nc"] = _build(plan)
        _CACHE["key"] = key
    nc = _CACHE["nc"]

    from concourse.bass_utils import run_bass_kernel_spmd

    trace = bool(os.environ.get("PINGNN_TRACE"))
    res = run_bass_kernel_spmd(nc, in_maps, list(range(NC)), trace=trace)
    _CACHE["last_result"] = res
    return np.asarray(res.results[0]["out"], F32)
